# revision 1
# baseline (speedup 1.0000x reference)
"""GCNConv kernel for 8 Trainium2 NeuronCores (Bass/Tile).

Computes out = segment_sum(edge_val * (x @ W)[edge_col], edge_row) + b
as out = (A @ x) @ W + b  (associativity), with:
  - nodes (rows of output) sharded across 8 cores (12500 each)
  - edges partitioned by destination row -> per-core, per-128-row-tile
  - per 128-edge block: gather x[col] rows (fp16, 512B) via dma_gather,
    build a one-hot selection matrix S[e, dloc[e]] = val[e] with a single
    fused DVE tensor_scalar (is_equal x mult), and accumulate
    z[128 nodes, 256] += S.T @ X_block on the PE in PSUM.
  - epilogue per tile: transpose z, project by W (fp16), add bias, store.

x is split into 4 banks of 25000 rows because dma_gather indices are int16.
"""
import os
from contextlib import ExitStack

import numpy as np

import concourse.bass as bass
import concourse.tile as tile
from concourse import bacc, mybir
from concourse.bass_utils import run_bass_kernel_spmd

P = 128
D = 256
N_NODES = 100000
N_EDGES = 3200000
NC = 8
SH = N_NODES // NC          # 12500 rows per core
NT = (SH + P - 1) // P      # 98 tiles per core
NBANK = 4
BS = N_NODES // NBANK       # 25000 rows per bank (fits int16 index)

F16 = mybir.dt.float16
F32 = mybir.dt.float32
I16 = mybir.dt.int16

_last_results = None        # BassKernelResults of the most recent run


def _build_structure(edge_row, edge_col, edge_val):
    """Sort/pad edges into per-core 128-edge blocks grouped by
    (dest tile, source bank).  Block structure (nb_tk) is shared across
    cores (padded to the max) so one SPMD program fits all cores.

    Returns (nb_tk [NT,NBANK] int, per-core dict arrays).
    """
    E = edge_row.shape[0]
    core = edge_row // SH
    r_loc = edge_row - core * SH
    t = r_loc // P
    dloc = (r_loc % P).astype(np.float32)
    bank = edge_col // BS
    bidx = (edge_col % BS).astype(np.int16)

    gid = (core.astype(np.int64) * NT + t) * NBANK + bank
    order = np.argsort(gid, kind="stable")
    gid_s = gid[order]

    cnt = np.bincount(gid, minlength=NC * NT * NBANK).reshape(NC, NT, NBANK)
    nb_tk = (cnt.max(axis=0) + P - 1) // P          # [NT, NBANK] blocks
    nb_tk = np.maximum(nb_tk, 1)                     # keep structure non-empty
    NB_t = nb_tk.sum(axis=1)                         # [NT]
    NBLK = int(NB_t.sum())
    pad_len = NBLK * P

    # slot offset of group (t,k) within a core's padded edge list
    off_tk = np.zeros((NT, NBANK), np.int64)
    flat_off = np.cumsum(nb_tk.ravel() * P)
    off_tk.ravel()[1:] = flat_off[:-1]

    # position of each edge within its (c,t,k) group
    grp_start = np.zeros(E, np.int64)
    newgrp = np.ones(E, bool)
    newgrp[1:] = gid_s[1:] != gid_s[:-1]
    starts = np.where(newgrp)[0]
    grp_start[starts] = starts
    grp_start = np.maximum.accumulate(grp_start)
    pos_in_grp = np.arange(E) - grp_start

    tk_of_edge = gid_s % (NT * NBANK)
    core_of_edge = gid_s // (NT * NBANK)
    dest = off_tk.ravel()[tk_of_edge] + pos_in_grp

    cores = []
    ev32 = edge_val.astype(np.float32)
    for c in range(NC):
        m = core_of_edge == c
        e_ids = order[m]
        d = dest[m]
        idx_arr = np.zeros(pad_len, np.int16)
        dloc_arr = np.zeros(pad_len, np.float32)
        val_arr = np.zeros(pad_len, np.float32)
        idx_arr[d] = bidx[e_ids]
        dloc_arr[d] = dloc[e_ids]
        val_arr[d] = ev32[e_ids]

        # packed gather indices: [128, 8*NBLK] int16 (16-wrap, replicated x8)
        idxp = np.tile(np.ascontiguousarray(idx_arr.reshape(-1, 16).T), (8, 1))
        # per-block [dloc, val, -dloc, -val]: [128, 4*NBLK] f32
        dl = np.ascontiguousarray(dloc_arr.reshape(NBLK, P).T)
        vv = np.ascontiguousarray(val_arr.reshape(NBLK, P).T)
        dv = np.empty((P, 4 * NBLK), np.float32)
        dv[:, 0::4] = dl
        dv[:, 1::4] = vv
        dv[:, 2::4] = -dl
        dv[:, 3::4] = -vv
        cores.append(dict(idxp=idxp, dv=dv))

    return nb_tk, cores


def _build_program(nb_tk):
    """Build the SPMD Bass program for the given block structure."""
    nb_tk = np.asarray(nb_tk)
    NB_t = nb_tk.sum(axis=1)
    NBLK = int(NB_t.sum())
    nt = nb_tk.shape[0]
    out_rows = nt * P

    nc = bacc.Bacc("TRN2", target_bir_lowering=False, debug=False,
                   num_devices=NC, num_swdge_queues=4)
    xb_aps = [nc.dram_tensor(f"xb{k}", [BS, D], F16,
                             kind="ExternalInput").ap() for k in range(NBANK)]
    idxp_ap = nc.dram_tensor("idxp", [P, 8 * NBLK], I16,
                             kind="ExternalInput").ap()
    dv_ap = nc.dram_tensor("dv", [P, 4 * NBLK], F32,
                           kind="ExternalInput").ap()
    w_ap = nc.dram_tensor("w", [D, D], F16, kind="ExternalInput").ap()
    bias_ap = nc.dram_tensor("bias", [P, D], F32, kind="ExternalInput").ap()
    iota_ap = nc.dram_tensor("iota", [P, P], F16, kind="ExternalInput").ap()
    ident_ap = nc.dram_tensor("ident", [P, P], F16, kind="ExternalInput").ap()
    out_ap = nc.dram_tensor("out", [out_rows, D], F32,
                            kind="ExternalOutput").ap()

    nb_max = int(NB_t.max())

    with tile.TileContext(nc) as tc:
        with ExitStack() as ctx:
            const = ctx.enter_context(tc.tile_pool(name="const", bufs=1))
            idxpool = ctx.enter_context(tc.tile_pool(name="idxp", bufs=6))
            dvpool = ctx.enter_context(tc.tile_pool(name="dvp", bufs=6))
            xgpool = ctx.enter_context(tc.tile_pool(name="xgp", bufs=4))
            spool = ctx.enter_context(tc.tile_pool(name="sp", bufs=8))
            epool = ctx.enter_context(tc.tile_pool(name="ep", bufs=2))
            apool = ctx.enter_context(tc.tile_pool(name="ap", bufs=8))
            zpsum = ctx.enter_context(
                tc.tile_pool(name="zps", bufs=2, space="PSUM"))
            tpsum = ctx.enter_context(
                tc.tile_pool(name="tps", bufs=2, space="PSUM"))
            opsum = ctx.enter_context(
                tc.tile_pool(name="ops", bufs=2, space="PSUM"))

            iota_t = const.tile([P, P], F16, tag="iota")
            nc.sync.dma_start(iota_t[:], iota_ap[:])
            ident_t = const.tile([P, P], F16, tag="ident")
            nc.sync.dma_start(ident_t[:], ident_ap[:])
            w_t = const.tile([P, 2, D], F16, tag="w")
            nc.sync.dma_start(w_t[:], w_ap[:].rearrange("(c k) d -> k c d",
                                                        k=P))
            bias_t = const.tile([P, D], F32, tag="bias")
            nc.sync.dma_start(bias_t[:], bias_ap[:])

            TGL = 4  # tiles per idx/dv load group (prefetch + fewer sync ops)
            grp_nb_max = max(int(NB_t[g:g + TGL].sum())
                             for g in range(0, nt, TGL))
            bo = 0  # global block offset
            idx_t = dv_t = None
            gbo = 0  # block offset of current group start
            for t in range(nt):
                nb = int(NB_t[t])
                if t % TGL == 0:
                    gnb = int(NB_t[t:t + TGL].sum())
                    gbo = bo
                    idx_t = idxpool.tile([P, 8 * grp_nb_max], I16, tag="idx")
                    nc.sync.dma_start(idx_t[:, :8 * gnb],
                                      idxp_ap[:, 8 * bo:8 * (bo + gnb)])
                    dv_t = dvpool.tile([P, 4 * grp_nb_max], F32, tag="dv")
                    nc.sync.dma_start(dv_t[:, :4 * gnb],
                                      dv_ap[:, 4 * bo:4 * (bo + gnb)])
                lo = bo - gbo   # tile's block offset within the group tiles
                xg = xgpool.tile([P, nb_max, D], F16, tag="xg")
                ok = 0
                for k in range(NBANK):
                    nbk = int(nb_tk[t, k])
                    if nbk == 0:
                        continue
                    n = nbk * P
                    nc.gpsimd.dma_gather(
                        out_ap=xg[:, ok:ok + nbk, :],
                        in_ap=xb_aps[k][:],
                        idxs_ap=idx_t[:, 8 * (lo + ok):8 * (lo + ok + nbk)],
                        num_idxs=n,
                        num_idxs_reg=n,
                        elem_size=D,
                        # >64 descriptors (~1008 idxs) break the one-packet
                        # ceiling and wedge the exec unit
                        single_packet=(n <= 992),
                        queue_num=k,
                    )
                    ok += nbk

                z_ps = zpsum.tile([P, D], F32, tag="zps")
                for j in range(nb):
                    c = 4 * (lo + j)
                    s_t = spool.tile([P, P], F16, tag="s")
                    if (bo + j) % 10 < 3:
                        # offload ~30% of S-builds to the idle scalar engine:
                        # S = relu(val - val*|iota - dloc|)
                        a_t = apool.tile([P, P], F16, tag="at")
                        nc.scalar.activation(
                            a_t[:], iota_t[:],
                            mybir.ActivationFunctionType.Abs,
                            bias=dv_t[:, c + 2:c + 3])
                        nc.scalar.activation(
                            s_t[:], a_t[:],
                            mybir.ActivationFunctionType.Relu,
                            scale=dv_t[:, c + 3:c + 4],
                            bias=dv_t[:, c + 1:c + 2])
                    else:
                        nc.vector.tensor_scalar(
                            out=s_t[:], in0=iota_t[:],
                            scalar1=dv_t[:, c:c + 1],
                            scalar2=dv_t[:, c + 1:c + 2],
                            op0=mybir.AluOpType.is_equal,
                            op1=mybir.AluOpType.mult,
                        )
                    nc.tensor.matmul(out=z_ps[:], lhsT=s_t[:],
                                     rhs=xg[:, j, :],
                                     start=(j == 0), stop=(j == nb - 1))

                z_sb = epool.tile([P, D], F16, tag="zsb")
                nc.scalar.copy(z_sb[:], z_ps[:])
                o_ps = opsum.tile([P, D], F32, tag="ops")
                for ch in range(2):
                    zt_ps = tpsum.tile([P, P], F16, tag="ztps")
                    nc.tensor.transpose(zt_ps[:],
                                        z_sb[:, ch * P:(ch + 1) * P],
                                        ident_t[:])
                    zt_sb = epool.tile([P, P], F16, tag="ztsb")
                    nc.scalar.copy(zt_sb[:], zt_ps[:])
                    nc.tensor.matmul(out=o_ps[:], lhsT=zt_sb[:],
                                     rhs=w_t[:, ch, :],
                                     start=(ch == 0), stop=(ch == 1))
                o_sb = epool.tile([P, D], F32, tag="osb")
                nc.vector.tensor_add(o_sb[:], o_ps[:], bias_t[:])
                nc.sync.dma_start(out_ap[t * P:(t + 1) * P, :], o_sb[:])
                bo += nb
    nc.compile()
    return nc


def kernel(x, edge_row, edge_col, edge_val, weight, b):
    global _last_results
    assert x.shape == (N_NODES, D)

    nb_tk, cores = _build_structure(
        np.asarray(edge_row), np.asarray(edge_col), np.asarray(edge_val))
    nc = _build_program(nb_tk)

    x16 = np.asarray(x, np.float32).astype(np.float16)
    banks = [np.ascontiguousarray(x16[k * BS:(k + 1) * BS])
             for k in range(NBANK)]
    w16 = np.asarray(weight, np.float32).astype(np.float16)
    bias = np.broadcast_to(
        np.asarray(b, np.float32)[None, :], (P, D)).copy()
    iota = np.tile(np.arange(P, dtype=np.float16)[None, :], (P, 1))
    ident = np.eye(P, dtype=np.float16)

    in_maps = []
    for c in range(NC):
        m = {f"xb{k}": banks[k] for k in range(NBANK)}
        m.update(idxp=cores[c]["idxp"], dv=cores[c]["dv"], w=w16,
                 bias=bias, iota=iota, ident=ident)
        in_maps.append(m)

    trace = bool(os.environ.get("KERNEL_TRACE"))
    res = run_bass_kernel_spmd(nc, in_maps, list(range(NC)), trace=trace)
    _last_results = res

    out = np.concatenate([res.results[c]["out"][:SH] for c in range(NC)],
                         axis=0)
    return out.astype(np.float32)



# revision 7
# speedup vs baseline: 1.2353x; 1.2353x over previous
"""GCNConv kernel for 8 Trainium2 NeuronCores (Bass/Tile).

Computes out = segment_sum(edge_val * (x @ W)[edge_col], edge_row) + b
as out = (A @ x) @ W + b  (associativity), with:
  - nodes (rows of output) sharded across 8 cores (12500 each)
  - edges partitioned by destination row -> per-core, per-128-row-tile
  - per 128-edge block: gather x[col] rows (fp16, 512B) via dma_gather.
    Gathers for the 4 source banks use independent idx tensors and
    output tiles on 4 SWDGE queues so the Tile scheduler cannot
    serialize them.
  - the one-hot scatter matrices S for a whole tile are built with TWO
    batched DVE tensor_tensor ops (is_equal then mult) over a wide
    [128, nb*128] tile.  The comparison operand (dloc) and scale (val)
    are host-replicated x8 so the innermost AP dim stays packed, which
    keeps the DVE in its 2x 16-bit perf mode.
  - z[128 nodes, 256] += S_j.T @ X_j accumulated on the PE in PSUM.
  - epilogue per tile: transpose z, project by W (fp16), bias via a
    PSUM-initializing ident @ bias matmul, store.  All PSUM->SBUF
    copies ride the scalar (ACT) engine; the DVE only does the two
    batched builds per tile.

x is split into 4 banks of 25000 rows because dma_gather indices are
int16.  Padding slots use idx=0 with dloc=255 so their one-hot column
stays zero.
"""
import os
from contextlib import ExitStack

import numpy as np

import concourse.bass as bass
import concourse.tile as tile
from concourse import bacc, mybir
from concourse.bass_utils import run_bass_kernel_spmd

P = 128
D = 256
N_NODES = 100000
N_EDGES = 3200000
NC = 8
SH = N_NODES // NC          # 12500 rows per core
NT = (SH + P - 1) // P      # 98 tiles per core
NBANK = 4
BS = N_NODES // NBANK       # 25000 rows per bank (fits int16 index)

F16 = mybir.dt.float16
F32 = mybir.dt.float32
I16 = mybir.dt.int16

_last_results = None        # BassKernelResults of the most recent run


def _build_structure(edge_row, edge_col, edge_val):
    """Sort/pad edges into per-core 128-edge blocks grouped by
    (dest tile, source bank).  Block structure (nb_tk, v_tk) is shared
    across cores (padded to the max) so one SPMD program fits all cores.

    Returns (nb_tk [NT,NBANK], v_tk [NT,NBANK], per-core dict arrays).
    """
    E = edge_row.shape[0]
    core = edge_row // SH
    r_loc = edge_row - core * SH
    t = r_loc // P
    dloc = (r_loc % P).astype(np.float16)
    bank = edge_col // BS
    bidx = (edge_col % BS).astype(np.int16)

    gid = (core.astype(np.int64) * NT + t) * NBANK + bank
    # secondary sort key: source index, for HBM locality within a group
    order = np.lexsort((bidx, gid))
    gid_s = gid[order]

    cnt = np.bincount(gid, minlength=NC * NT * NBANK).reshape(NC, NT, NBANK)
    v_tk = cnt.max(axis=0)                           # [NT, NBANK] valid idxs
    nb_tk = (v_tk + P - 1) // P                      # [NT, NBANK] blocks
    nb_tk = np.maximum(nb_tk, 1)                     # keep structure non-empty
    NB_t = nb_tk.sum(axis=1)                         # [NT]
    NBLK = int(NB_t.sum())
    pad_len = NBLK * P

    # slot offset of group (t,k) within a core's padded edge list
    off_tk = np.zeros((NT, NBANK), np.int64)
    flat_off = np.cumsum(nb_tk.ravel() * P)
    off_tk.ravel()[1:] = flat_off[:-1]

    # position of each edge within its (c,t,k) group
    grp_start = np.zeros(E, np.int64)
    newgrp = np.ones(E, bool)
    newgrp[1:] = gid_s[1:] != gid_s[:-1]
    starts = np.where(newgrp)[0]
    grp_start[starts] = starts
    grp_start = np.maximum.accumulate(grp_start)
    pos_in_grp = np.arange(E) - grp_start

    tk_of_edge = gid_s % (NT * NBANK)
    core_of_edge = gid_s // (NT * NBANK)
    dest = off_tk.ravel()[tk_of_edge] + pos_in_grp

    # per-bank block bookkeeping: for tile t, bank k, blocks occupy
    # [bko_tk[t,k], bko_tk[t,k]+nb_tk[t,k]) within that bank's block list
    bko_tk = np.zeros((NT, NBANK), np.int64)
    for k in range(NBANK):
        bko_tk[1:, k] = np.cumsum(nb_tk[:-1, k])
    NBLK_k = nb_tk.sum(axis=0)                       # [NBANK]

    # map global block -> (bank, per-bank block id)
    bank_of_blk = np.concatenate(
        [np.repeat(np.arange(NBANK), nb_tk[t]) for t in range(NT)])

    cores = []
    ev16 = edge_val.astype(np.float16)
    for c in range(NC):
        m = core_of_edge == c
        e_ids = order[m]
        d = dest[m]
        idx_arr = np.zeros(pad_len, np.int16)
        dloc_arr = np.full(pad_len, 255.0, np.float16)  # pad: no one-hot hit
        val_arr = np.zeros(pad_len, np.float16)
        idx_arr[d] = bidx[e_ids]
        dloc_arr[d] = dloc[e_ids]
        val_arr[d] = ev16[e_ids]
        # per-bank packed gather indices: [128, 8*NBLK_k] int16
        idxps = []
        blk_idx = idx_arr.reshape(NBLK, P)
        for k in range(NBANK):
            sel = blk_idx[bank_of_blk == k].ravel()
            idxps.append(np.tile(
                np.ascontiguousarray(sel.reshape(-1, 16).T), (8, 1)))
        # dv8: per block [dl x8 | val x8]: [128, NBLK, 2, 8] f16
        dl = np.ascontiguousarray(dloc_arr.reshape(NBLK, P).T)  # [128, NBLK]
        vv = np.ascontiguousarray(val_arr.reshape(NBLK, P).T)
        dv8 = np.empty((P, NBLK, 2, 8), np.float16)
        dv8[:, :, 0, :] = dl[:, :, None]
        dv8[:, :, 1, :] = vv[:, :, None]
        cores.append(dict(idxps=idxps, dv8=dv8.reshape(P, NBLK * 16)))

    return nb_tk, v_tk, bko_tk, NBLK_k, cores


def _build_program(nb_tk, v_tk, bko_tk, NBLK_k):
    """Build the SPMD Bass program for the given block structure."""
    nb_tk = np.asarray(nb_tk)
    NB_t = nb_tk.sum(axis=1)
    NBLK = int(NB_t.sum())
    nt = nb_tk.shape[0]
    out_rows = nt * P

    nc = bacc.Bacc("TRN2", target_bir_lowering=False, debug=False,
                   num_devices=NC, num_swdge_queues=4)
    xb_aps = [nc.dram_tensor(f"xb{k}", [BS, D], F16,
                             kind="ExternalInput").ap() for k in range(NBANK)]
    idxp_aps = [nc.dram_tensor(f"idxp{k}", [P, 8 * int(NBLK_k[k])], I16,
                               kind="ExternalInput").ap()
                for k in range(NBANK)]
    dv8_ap = nc.dram_tensor("dv8", [P, 16 * NBLK], F16,
                            kind="ExternalInput").ap()
    w_ap = nc.dram_tensor("w", [D, D], F16, kind="ExternalInput").ap()
    bias_ap = nc.dram_tensor("bias", [P, D], F16, kind="ExternalInput").ap()
    iota_ap = nc.dram_tensor("iota", [P, P], F16, kind="ExternalInput").ap()
    ident_ap = nc.dram_tensor("ident", [P, P], F16, kind="ExternalInput").ap()
    out_ap = nc.dram_tensor("out", [out_rows, D], F32,
                            kind="ExternalOutput").ap()

    nb_max = int(NB_t.max())
    nbk_max = [int(nb_tk[:, k].max()) for k in range(NBANK)]

    with tile.TileContext(nc) as tc:
        with ExitStack() as ctx:
            const = ctx.enter_context(tc.tile_pool(name="const", bufs=1))
            idxpools = [ctx.enter_context(
                tc.tile_pool(name=f"idxp{k}", bufs=4)) for k in range(NBANK)]
            dvpool = ctx.enter_context(tc.tile_pool(name="dvp", bufs=4))
            xgpools = [ctx.enter_context(
                tc.tile_pool(name=f"xgp{k}", bufs=4)) for k in range(NBANK)]
            swpool = ctx.enter_context(tc.tile_pool(name="swp", bufs=3))
            epool = ctx.enter_context(tc.tile_pool(name="ep", bufs=2))
            zpsum = ctx.enter_context(
                tc.tile_pool(name="zps", bufs=2, space="PSUM"))
            tpsum = ctx.enter_context(
                tc.tile_pool(name="tps", bufs=2, space="PSUM"))
            opsum = ctx.enter_context(
                tc.tile_pool(name="ops", bufs=2, space="PSUM"))

            iota_t = const.tile([P, P], F16, tag="iota")
            nc.sync.dma_start(iota_t[:], iota_ap[:])
            ident_t = const.tile([P, P], F16, tag="ident")
            nc.sync.dma_start(ident_t[:], ident_ap[:])
            w_t = const.tile([P, 2, D], F16, tag="w")
            nc.sync.dma_start(w_t[:], w_ap[:].rearrange("(c k) d -> k c d",
                                                        k=P))
            bias_t = const.tile([P, D], F16, tag="bias")
            nc.sync.dma_start(bias_t[:], bias_ap[:])

            TGL = 4  # tiles per idx/dv load group (prefetch + fewer sync ops)
            grp_nb_max = max(int(NB_t[g:g + TGL].sum())
                             for g in range(0, nt, TGL))
            grp_nbk_max = [
                max(int(nb_tk[g:g + TGL, k].sum()) for g in range(0, nt, TGL))
                for k in range(NBANK)]
            bo = 0   # global block offset
            gbo = 0  # block offset of current group start
            dv_t = None
            idx_ts = [None] * NBANK
            gko = [0] * NBANK  # per-bank block offset of group start
            for t in range(nt):
                nb = int(NB_t[t])
                if t % TGL == 0:
                    gnb = int(NB_t[t:t + TGL].sum())
                    gbo = bo
                    for k in range(NBANK):
                        gnbk = int(nb_tk[t:t + TGL, k].sum())
                        gko[k] = int(bko_tk[t, k])
                        idx_ts[k] = idxpools[k].tile(
                            [P, 8 * grp_nbk_max[k]], I16, tag=f"idx{k}",
                            name=f"idx{k}")
                        nc.sync.dma_start(
                            idx_ts[k][:, :8 * gnbk],
                            idxp_aps[k][:, 8 * gko[k]:8 * (gko[k] + gnbk)])
                    dv_t = dvpool.tile([P, 16 * grp_nb_max], F16, tag="dv")
                    nc.sync.dma_start(dv_t[:, :16 * gnb],
                                      dv8_ap[:, 16 * bo:16 * (bo + gnb)])
                lo = bo - gbo   # tile's block offset within the group tiles
                xgs = []
                for k in range(NBANK):
                    nbk = int(nb_tk[t, k])
                    xg = xgpools[k].tile([P, nbk_max[k], D], F16,
                                         tag=f"xg{k}", name=f"xg{k}")
                    xgs.append(xg)
                    n = nbk * P
                    klo = int(bko_tk[t, k]) - gko[k]
                    nc.gpsimd.dma_gather(
                        out_ap=xg[:, 0:nbk, :],
                        in_ap=xb_aps[k][:],
                        idxs_ap=idx_ts[k][:, 8 * klo:8 * (klo + nbk)],
                        num_idxs=n,
                        num_idxs_reg=n,
                        elem_size=D,
                        # >64 descriptors (~1008 idxs) break the one-packet
                        # ceiling and wedge the exec unit
                        single_packet=(n <= 992),
                        queue_num=k,
                    )

                # batched one-hot build: S[p, j*128+d] = val_j[p] *
                # (iota[d] == dloc_j[p]); two wide DVE tensor_tensor ops.
                s_wide = swpool.tile([P, nb_max * P], F16, tag="sw")
                dv_g = dv_t[:, 16 * lo:16 * (lo + nb)].rearrange(
                    "p (j c r) -> p j c r", c=2, r=8)
                io_b = (iota_t[:].rearrange("p (q r) -> p q r", r=8)
                        .unsqueeze(1).broadcast_to([P, nb, 16, 8]))
                dl_b = dv_g[:, :, 0:1, :].broadcast_to([P, nb, 16, 8])
                vl_b = dv_g[:, :, 1:2, :].broadcast_to([P, nb, 16, 8])
                sw4 = s_wide[:, :nb * P].rearrange(
                    "p (j q r) -> p j q r", q=16, r=8)
                nc.vector.tensor_tensor(out=sw4, in0=io_b, in1=dl_b,
                                        op=mybir.AluOpType.is_equal)
                nc.vector.tensor_tensor(out=sw4, in0=sw4, in1=vl_b,
                                        op=mybir.AluOpType.mult)

                z_ps = zpsum.tile([P, D], F32, tag="zps")
                j = 0
                for k in range(NBANK):
                    nbk = int(nb_tk[t, k])
                    for jl in range(nbk):
                        nc.tensor.matmul(out=z_ps[:],
                                         lhsT=s_wide[:, j * P:(j + 1) * P],
                                         rhs=xgs[k][:, jl, :],
                                         start=(j == 0), stop=(j == nb - 1))
                        j += 1

                z_sb = epool.tile([P, D], F16, tag="zsb")
                nc.scalar.copy(z_sb[:], z_ps[:])
                o_ps = opsum.tile([P, D], F32, tag="ops")
                # seed o_ps with the bias via ident.T @ bias_tile
                nc.tensor.matmul(out=o_ps[:], lhsT=ident_t[:], rhs=bias_t[:],
                                 start=True, stop=False)
                for ch in range(2):
                    zt_ps = tpsum.tile([P, P], F16, tag="ztps")
                    nc.tensor.transpose(zt_ps[:],
                                        z_sb[:, ch * P:(ch + 1) * P],
                                        ident_t[:])
                    zt_sb = epool.tile([P, P], F16, tag="ztsb")
                    nc.scalar.copy(zt_sb[:], zt_ps[:])
                    nc.tensor.matmul(out=o_ps[:], lhsT=zt_sb[:],
                                     rhs=w_t[:, ch, :],
                                     start=False, stop=(ch == 1))
                o_sb = epool.tile([P, D], F32, tag="osb")
                nc.scalar.copy(o_sb[:], o_ps[:])
                nc.sync.dma_start(out_ap[t * P:(t + 1) * P, :], o_sb[:])
                bo += nb
    # The Tile scheduler assigns SWDGE completion sems (DMASW lanes) round-
    # robin in *scheduled* order, and the runtime locks each sem to one
    # SWDGE queue.  Align queue_num with the assigned lane.
    _DMASW0 = 11  # PROC_NAME_TO_IDX["DMASW0"]
    for inst in nc.all_instructions():
        if type(inst).__name__ == "InstDMAGatherAnt":
            inst.queue_num = (inst.bass_scheduled_proc - _DMASW0) % 4
    nc.compile()
    return nc


def kernel(x, edge_row, edge_col, edge_val, weight, b):
    global _last_results
    assert x.shape == (N_NODES, D)

    nb_tk, v_tk, bko_tk, NBLK_k, cores = _build_structure(
        np.asarray(edge_row), np.asarray(edge_col), np.asarray(edge_val))
    nc = _build_program(nb_tk, v_tk, bko_tk, NBLK_k)

    x16 = np.asarray(x, np.float32).astype(np.float16)
    banks = [np.ascontiguousarray(x16[k * BS:(k + 1) * BS])
             for k in range(NBANK)]
    w16 = np.asarray(weight, np.float32).astype(np.float16)
    bias = np.broadcast_to(
        np.asarray(b, np.float32).astype(np.float16)[None, :], (P, D)).copy()
    iota = np.tile(np.arange(P, dtype=np.float16)[None, :], (P, 1))
    ident = np.eye(P, dtype=np.float16)

    in_maps = []
    for c in range(NC):
        m = {f"xb{k}": banks[k] for k in range(NBANK)}
        m.update({f"idxp{k}": cores[c]["idxps"][k] for k in range(NBANK)})
        m.update(dv8=cores[c]["dv8"], w=w16, bias=bias, iota=iota,
                 ident=ident)
        in_maps.append(m)

    trace = bool(os.environ.get("KERNEL_TRACE"))
    res = run_bass_kernel_spmd(nc, in_maps, list(range(NC)), trace=trace)
    _last_results = res

    out = np.concatenate([res.results[c]["out"][:SH] for c in range(NC)],
                         axis=0)
    return out.astype(np.float32)


# revision 8
# speedup vs baseline: 1.2669x; 1.0256x over previous
"""GCNConv kernel for 8 Trainium2 NeuronCores (Bass/Tile).

Computes out = segment_sum(edge_val * (x @ W)[edge_col], edge_row) + b
as out = (A @ x) @ W + b  (associativity), with:
  - nodes (rows of output) sharded across 8 cores (12500 each)
  - edges partitioned by destination row -> per-core, per-128-row-tile
  - per 128-edge block: gather x[col] rows (fp16, 512B) via dma_gather.
    Gathers for the 4 source banks use independent idx tensors and
    output tiles on 4 SWDGE queues so the Tile scheduler cannot
    serialize them.
  - the one-hot scatter matrices S for a whole tile are built with TWO
    batched DVE tensor_tensor ops (is_equal then mult) over a wide
    [128, nb*128] tile.  The comparison operand (dloc) and scale (val)
    are host-replicated x8 so the innermost AP dim stays packed, which
    keeps the DVE in its 2x 16-bit perf mode.
  - z[128 nodes, 256] += S_j.T @ X_j accumulated on the PE in PSUM.
  - epilogue per tile: transpose z, project by W (fp16), bias via a
    PSUM-initializing ident @ bias matmul, store.  All PSUM->SBUF
    copies ride the scalar (ACT) engine; the DVE only does the two
    batched builds per tile.

x is split into 4 banks of 25000 rows because dma_gather indices are
int16.  Padding slots use idx=0 with dloc=255 so their one-hot column
stays zero.
"""
import os
from contextlib import ExitStack

import numpy as np

import concourse.bass as bass
import concourse.tile as tile
from concourse import bacc, mybir
from concourse.bass_utils import run_bass_kernel_spmd

P = 128
D = 256
N_NODES = 100000
N_EDGES = 3200000
NC = 8
SH = N_NODES // NC          # 12500 rows per core
NT = (SH + P - 1) // P      # 98 tiles per core
NBANK = 4
BS = N_NODES // NBANK       # 25000 rows per bank (fits int16 index)

F16 = mybir.dt.float16
F32 = mybir.dt.float32
I16 = mybir.dt.int16

_last_results = None        # BassKernelResults of the most recent run


def _build_structure(edge_row, edge_col, edge_val):
    """Sort/pad edges into per-core 128-edge blocks grouped by
    (dest tile, source bank).  Block structure (nb_tk, v_tk) is shared
    across cores (padded to the max) so one SPMD program fits all cores.

    Returns (nb_tk [NT,NBANK], v_tk [NT,NBANK], per-core dict arrays).
    """
    E = edge_row.shape[0]
    core = edge_row // SH
    r_loc = edge_row - core * SH
    t = r_loc // P
    dloc = (r_loc % P).astype(np.float16)
    bank = edge_col // BS
    bidx = (edge_col % BS).astype(np.int16)

    gid = (core.astype(np.int64) * NT + t) * NBANK + bank
    # secondary sort key: source index, for HBM locality within a group
    order = np.lexsort((bidx, gid))
    gid_s = gid[order]

    cnt = np.bincount(gid, minlength=NC * NT * NBANK).reshape(NC, NT, NBANK)
    v_tk = cnt.max(axis=0)                           # [NT, NBANK] valid idxs
    nb_tk = (v_tk + P - 1) // P                      # [NT, NBANK] blocks
    nb_tk = np.maximum(nb_tk, 1)                     # keep structure non-empty
    NB_t = nb_tk.sum(axis=1)                         # [NT]
    NBLK = int(NB_t.sum())
    pad_len = NBLK * P

    # slot offset of group (t,k) within a core's padded edge list
    off_tk = np.zeros((NT, NBANK), np.int64)
    flat_off = np.cumsum(nb_tk.ravel() * P)
    off_tk.ravel()[1:] = flat_off[:-1]

    # position of each edge within its (c,t,k) group
    grp_start = np.zeros(E, np.int64)
    newgrp = np.ones(E, bool)
    newgrp[1:] = gid_s[1:] != gid_s[:-1]
    starts = np.where(newgrp)[0]
    grp_start[starts] = starts
    grp_start = np.maximum.accumulate(grp_start)
    pos_in_grp = np.arange(E) - grp_start

    tk_of_edge = gid_s % (NT * NBANK)
    core_of_edge = gid_s // (NT * NBANK)
    dest = off_tk.ravel()[tk_of_edge] + pos_in_grp

    # per-bank block bookkeeping: for tile t, bank k, blocks occupy
    # [bko_tk[t,k], bko_tk[t,k]+nb_tk[t,k]) within that bank's block list
    bko_tk = np.zeros((NT, NBANK), np.int64)
    for k in range(NBANK):
        bko_tk[1:, k] = np.cumsum(nb_tk[:-1, k])
    NBLK_k = nb_tk.sum(axis=0)                       # [NBANK]

    # map global block -> (bank, per-bank block id)
    bank_of_blk = np.concatenate(
        [np.repeat(np.arange(NBANK), nb_tk[t]) for t in range(NT)])

    cores = []
    ev16 = edge_val.astype(np.float16)
    for c in range(NC):
        m = core_of_edge == c
        e_ids = order[m]
        d = dest[m]
        idx_arr = np.zeros(pad_len, np.int16)
        dloc_arr = np.full(pad_len, 255.0, np.float16)  # pad: no one-hot hit
        val_arr = np.zeros(pad_len, np.float16)
        idx_arr[d] = bidx[e_ids]
        dloc_arr[d] = dloc[e_ids]
        val_arr[d] = ev16[e_ids]
        # per-bank packed gather indices: [128, 8*NBLK_k] int16
        idxps = []
        blk_idx = idx_arr.reshape(NBLK, P)
        for k in range(NBANK):
            sel = blk_idx[bank_of_blk == k].ravel()
            idxps.append(np.tile(
                np.ascontiguousarray(sel.reshape(-1, 16).T), (8, 1)))
        # dv8: per block [dl x8 | val x8]: [128, NBLK, 2, 8] f16
        dl = np.ascontiguousarray(dloc_arr.reshape(NBLK, P).T)  # [128, NBLK]
        vv = np.ascontiguousarray(val_arr.reshape(NBLK, P).T)
        dv8 = np.empty((P, NBLK, 2, 8), np.float16)
        dv8[:, :, 0, :] = dl[:, :, None]
        dv8[:, :, 1, :] = vv[:, :, None]
        cores.append(dict(idxps=idxps, dv8=dv8.reshape(P, NBLK * 16)))

    return nb_tk, v_tk, bko_tk, NBLK_k, cores


def _build_program(nb_tk, v_tk, bko_tk, NBLK_k):
    """Build the SPMD Bass program for the given block structure."""
    nb_tk = np.asarray(nb_tk)
    NB_t = nb_tk.sum(axis=1)
    NBLK = int(NB_t.sum())
    nt = nb_tk.shape[0]
    out_rows = nt * P

    nc = bacc.Bacc("TRN2", target_bir_lowering=False, debug=False,
                   num_devices=NC, num_swdge_queues=4)
    xb_aps = [nc.dram_tensor(f"xb{k}", [BS, D], F16,
                             kind="ExternalInput").ap() for k in range(NBANK)]
    idxp_aps = [nc.dram_tensor(f"idxp{k}", [P, 8 * int(NBLK_k[k])], I16,
                               kind="ExternalInput").ap()
                for k in range(NBANK)]
    dv8_ap = nc.dram_tensor("dv8", [P, 16 * NBLK], F16,
                            kind="ExternalInput").ap()
    w_ap = nc.dram_tensor("w", [D, D], F16, kind="ExternalInput").ap()
    bias_ap = nc.dram_tensor("bias", [P, D], F16, kind="ExternalInput").ap()
    iota_ap = nc.dram_tensor("iota", [P, P], F16, kind="ExternalInput").ap()
    ident_ap = nc.dram_tensor("ident", [P, P], F16, kind="ExternalInput").ap()
    out_ap = nc.dram_tensor("out", [out_rows, D], F32,
                            kind="ExternalOutput").ap()

    nb_max = int(NB_t.max())
    nbk_max = [int(nb_tk[:, k].max()) for k in range(NBANK)]

    with tile.TileContext(nc) as tc:
        with ExitStack() as ctx:
            const = ctx.enter_context(tc.tile_pool(name="const", bufs=1))
            idxpools = [ctx.enter_context(
                tc.tile_pool(name=f"idxp{k}", bufs=4)) for k in range(NBANK)]
            dvpool = ctx.enter_context(tc.tile_pool(name="dvp", bufs=4))
            xgpools = [ctx.enter_context(
                tc.tile_pool(name=f"xgp{k}", bufs=6)) for k in range(NBANK)]
            swpool = ctx.enter_context(tc.tile_pool(name="swp", bufs=4))
            epool = ctx.enter_context(tc.tile_pool(name="ep", bufs=2))
            zpsum = ctx.enter_context(
                tc.tile_pool(name="zps", bufs=4, space="PSUM"))
            tpsum = ctx.enter_context(
                tc.tile_pool(name="tps", bufs=2, space="PSUM"))
            opsum = ctx.enter_context(
                tc.tile_pool(name="ops", bufs=2, space="PSUM"))

            iota_t = const.tile([P, P], F16, tag="iota")
            nc.sync.dma_start(iota_t[:], iota_ap[:])
            ident_t = const.tile([P, P], F16, tag="ident")
            nc.sync.dma_start(ident_t[:], ident_ap[:])
            w_t = const.tile([P, 2, D], F16, tag="w")
            nc.sync.dma_start(w_t[:], w_ap[:].rearrange("(c k) d -> k c d",
                                                        k=P))
            bias_t = const.tile([P, D], F16, tag="bias")
            nc.sync.dma_start(bias_t[:], bias_ap[:])

            TGL = 4  # tiles per idx/dv load group (prefetch + fewer sync ops)
            grp_nb_max = max(int(NB_t[g:g + TGL].sum())
                             for g in range(0, nt, TGL))
            grp_nbk_max = [
                max(int(nb_tk[g:g + TGL, k].sum()) for g in range(0, nt, TGL))
                for k in range(NBANK)]
            bo = 0   # global block offset
            gbo = 0  # block offset of current group start
            dv_t = None
            idx_ts = [None] * NBANK
            gko = [0] * NBANK  # per-bank block offset of group start
            for t in range(nt):
                nb = int(NB_t[t])
                if t % TGL == 0:
                    gnb = int(NB_t[t:t + TGL].sum())
                    gbo = bo
                    for k in range(NBANK):
                        gnbk = int(nb_tk[t:t + TGL, k].sum())
                        gko[k] = int(bko_tk[t, k])
                        idx_ts[k] = idxpools[k].tile(
                            [P, 8 * grp_nbk_max[k]], I16, tag=f"idx{k}",
                            name=f"idx{k}")
                        nc.sync.dma_start(
                            idx_ts[k][:, :8 * gnbk],
                            idxp_aps[k][:, 8 * gko[k]:8 * (gko[k] + gnbk)])
                    dv_t = dvpool.tile([P, 16 * grp_nb_max], F16, tag="dv")
                    nc.sync.dma_start(dv_t[:, :16 * gnb],
                                      dv8_ap[:, 16 * bo:16 * (bo + gnb)])
                lo = bo - gbo   # tile's block offset within the group tiles
                xgs = []
                for k in range(NBANK):
                    nbk = int(nb_tk[t, k])
                    xg = xgpools[k].tile([P, nbk_max[k], D], F16,
                                         tag=f"xg{k}", name=f"xg{k}")
                    xgs.append(xg)
                    n = nbk * P
                    klo = int(bko_tk[t, k]) - gko[k]
                    nc.gpsimd.dma_gather(
                        out_ap=xg[:, 0:nbk, :],
                        in_ap=xb_aps[k][:],
                        idxs_ap=idx_ts[k][:, 8 * klo:8 * (klo + nbk)],
                        num_idxs=n,
                        num_idxs_reg=n,
                        elem_size=D,
                        # >64 descriptors (~1008 idxs) break the one-packet
                        # ceiling and wedge the exec unit
                        single_packet=(n <= 992),
                        queue_num=k,
                    )

                # batched one-hot build: S[p, j*128+d] = val_j[p] *
                # (iota[d] == dloc_j[p]); two wide DVE tensor_tensor ops.
                s_wide = swpool.tile([P, nb_max * P], F16, tag="sw")
                dv_g = dv_t[:, 16 * lo:16 * (lo + nb)].rearrange(
                    "p (j c r) -> p j c r", c=2, r=8)
                io_b = (iota_t[:].rearrange("p (q r) -> p q r", r=8)
                        .unsqueeze(1).broadcast_to([P, nb, 16, 8]))
                dl_b = dv_g[:, :, 0:1, :].broadcast_to([P, nb, 16, 8])
                vl_b = dv_g[:, :, 1:2, :].broadcast_to([P, nb, 16, 8])
                sw4 = s_wide[:, :nb * P].rearrange(
                    "p (j q r) -> p j q r", q=16, r=8)
                nc.vector.tensor_tensor(out=sw4, in0=io_b, in1=dl_b,
                                        op=mybir.AluOpType.is_equal)
                nc.vector.tensor_tensor(out=sw4, in0=sw4, in1=vl_b,
                                        op=mybir.AluOpType.mult)

                z_ps = zpsum.tile([P, D], F32, tag="zps")
                j = 0
                for k in range(NBANK):
                    nbk = int(nb_tk[t, k])
                    for jl in range(nbk):
                        nc.tensor.matmul(out=z_ps[:],
                                         lhsT=s_wide[:, j * P:(j + 1) * P],
                                         rhs=xgs[k][:, jl, :],
                                         start=(j == 0), stop=(j == nb - 1))
                        j += 1

                z_sb = epool.tile([P, D], F16, tag="zsb")
                nc.scalar.copy(z_sb[:], z_ps[:])
                o_ps = opsum.tile([P, D], F32, tag="ops")
                # seed o_ps with the bias via ident.T @ bias_tile
                nc.tensor.matmul(out=o_ps[:], lhsT=ident_t[:], rhs=bias_t[:],
                                 start=True, stop=False)
                for ch in range(2):
                    zt_ps = tpsum.tile([P, P], F16, tag="ztps")
                    nc.tensor.transpose(zt_ps[:],
                                        z_sb[:, ch * P:(ch + 1) * P],
                                        ident_t[:])
                    zt_sb = epool.tile([P, P], F16, tag="ztsb")
                    nc.scalar.copy(zt_sb[:], zt_ps[:])
                    nc.tensor.matmul(out=o_ps[:], lhsT=zt_sb[:],
                                     rhs=w_t[:, ch, :],
                                     start=False, stop=(ch == 1))
                o_sb = epool.tile([P, D], F32, tag="osb")
                nc.scalar.copy(o_sb[:], o_ps[:])
                nc.sync.dma_start(out_ap[t * P:(t + 1) * P, :], o_sb[:])
                bo += nb
    # The Tile scheduler assigns SWDGE completion sems (DMASW lanes) round-
    # robin in *scheduled* order, and the runtime locks each sem to one
    # SWDGE queue.  Align queue_num with the assigned lane.
    _DMASW0 = 11  # PROC_NAME_TO_IDX["DMASW0"]
    for inst in nc.all_instructions():
        if type(inst).__name__ == "InstDMAGatherAnt":
            inst.queue_num = (inst.bass_scheduled_proc - _DMASW0) % 4
    nc.compile()
    return nc


def kernel(x, edge_row, edge_col, edge_val, weight, b):
    global _last_results
    assert x.shape == (N_NODES, D)

    nb_tk, v_tk, bko_tk, NBLK_k, cores = _build_structure(
        np.asarray(edge_row), np.asarray(edge_col), np.asarray(edge_val))
    nc = _build_program(nb_tk, v_tk, bko_tk, NBLK_k)

    x16 = np.asarray(x, np.float32).astype(np.float16)
    banks = [np.ascontiguousarray(x16[k * BS:(k + 1) * BS])
             for k in range(NBANK)]
    w16 = np.asarray(weight, np.float32).astype(np.float16)
    bias = np.broadcast_to(
        np.asarray(b, np.float32).astype(np.float16)[None, :], (P, D)).copy()
    iota = np.tile(np.arange(P, dtype=np.float16)[None, :], (P, 1))
    ident = np.eye(P, dtype=np.float16)

    in_maps = []
    for c in range(NC):
        m = {f"xb{k}": banks[k] for k in range(NBANK)}
        m.update({f"idxp{k}": cores[c]["idxps"][k] for k in range(NBANK)})
        m.update(dv8=cores[c]["dv8"], w=w16, bias=bias, iota=iota,
                 ident=ident)
        in_maps.append(m)

    trace = bool(os.environ.get("KERNEL_TRACE"))
    res = run_bass_kernel_spmd(nc, in_maps, list(range(NC)), trace=trace)
    _last_results = res

    out = np.concatenate([res.results[c]["out"][:SH] for c in range(NC)],
                         axis=0)
    return out.astype(np.float32)


# revision 9
# speedup vs baseline: 1.3936x; 1.1000x over previous
"""GCNConv kernel for 8 Trainium2 NeuronCores (Bass/Tile).

Computes out = segment_sum(edge_val * (x @ W)[edge_col], edge_row) + b
as out = (A @ x) @ W + b  (associativity), with:
  - nodes (rows of output) sharded across 8 cores (12500 each)
  - edges partitioned by destination row -> per-core, per-128-row-tile
  - per 128-edge block: gather x[col] rows (fp16, 512B) via dma_gather.
    Gathers for the 4 source banks use independent idx tensors and
    output tiles on 4 SWDGE queues so the Tile scheduler cannot
    serialize them.
  - the one-hot scatter matrices S for a whole tile are built with TWO
    batched DVE tensor_tensor ops (is_equal then mult) over a wide
    [128, nb*128] tile.  The comparison operand (dloc) and scale (val)
    are host-replicated x8 so the innermost AP dim stays packed, which
    keeps the DVE in its 2x 16-bit perf mode.
  - z[128 nodes, 256] += S_j.T @ X_j accumulated on the PE in PSUM.
  - epilogue per tile: transpose z, project by W (fp16), bias via a
    PSUM-initializing ident @ bias matmul, store.  All PSUM->SBUF
    copies ride the scalar (ACT) engine; the DVE only does the two
    batched builds per tile.

x is split into 4 banks of 25000 rows because dma_gather indices are
int16.  Padding slots use idx=0 with dloc=255 so their one-hot column
stays zero.
"""
import os
from contextlib import ExitStack

import numpy as np

import concourse.bass as bass
import concourse.tile as tile
from concourse import bacc, mybir
from concourse.bass_utils import run_bass_kernel_spmd

P = 128
D = 256
N_NODES = 100000
N_EDGES = 3200000
NC = 8
SH = N_NODES // NC          # 12500 rows per core
NT = (SH + P - 1) // P      # 98 tiles per core
NBANK = 4
BS = N_NODES // NBANK       # 25000 rows per bank (fits int16 index)

F16 = mybir.dt.float16
F32 = mybir.dt.float32
I16 = mybir.dt.int16

_last_results = None        # BassKernelResults of the most recent run


def _build_structure(edge_row, edge_col, edge_val):
    """Sort/pad edges into per-core 128-edge blocks grouped by
    (dest tile, source bank).  Block structure (nb_tk, v_tk) is shared
    across cores (padded to the max) so one SPMD program fits all cores.

    Returns (nb_tk [NT,NBANK], v_tk [NT,NBANK], per-core dict arrays).
    """
    E = edge_row.shape[0]
    core = edge_row // SH
    r_loc = edge_row - core * SH
    t = r_loc // P
    dloc = (r_loc % P).astype(np.float16)
    bank = edge_col // BS
    bidx = (edge_col % BS).astype(np.int16)

    gid = (core.astype(np.int64) * NT + t) * NBANK + bank
    # secondary sort key: source index, for HBM locality within a group
    order = np.lexsort((bidx, gid))
    gid_s = gid[order]

    cnt = np.bincount(gid, minlength=NC * NT * NBANK).reshape(NC, NT, NBANK)
    v_tk = cnt.max(axis=0)                           # [NT, NBANK] valid idxs
    nb_tk = (v_tk + P - 1) // P                      # [NT, NBANK] blocks
    nb_tk = np.maximum(nb_tk, 1)                     # keep structure non-empty
    NB_t = nb_tk.sum(axis=1)                         # [NT]
    NBLK = int(NB_t.sum())
    pad_len = NBLK * P

    # slot offset of group (t,k) within a core's padded edge list
    off_tk = np.zeros((NT, NBANK), np.int64)
    flat_off = np.cumsum(nb_tk.ravel() * P)
    off_tk.ravel()[1:] = flat_off[:-1]

    # position of each edge within its (c,t,k) group
    grp_start = np.zeros(E, np.int64)
    newgrp = np.ones(E, bool)
    newgrp[1:] = gid_s[1:] != gid_s[:-1]
    starts = np.where(newgrp)[0]
    grp_start[starts] = starts
    grp_start = np.maximum.accumulate(grp_start)
    pos_in_grp = np.arange(E) - grp_start

    tk_of_edge = gid_s % (NT * NBANK)
    core_of_edge = gid_s // (NT * NBANK)
    dest = off_tk.ravel()[tk_of_edge] + pos_in_grp

    # per-bank block bookkeeping: for tile t, bank k, blocks occupy
    # [bko_tk[t,k], bko_tk[t,k]+nb_tk[t,k]) within that bank's block list
    bko_tk = np.zeros((NT, NBANK), np.int64)
    for k in range(NBANK):
        bko_tk[1:, k] = np.cumsum(nb_tk[:-1, k])
    NBLK_k = nb_tk.sum(axis=0)                       # [NBANK]

    # map global block -> (bank, per-bank block id)
    bank_of_blk = np.concatenate(
        [np.repeat(np.arange(NBANK), nb_tk[t]) for t in range(NT)])

    cores = []
    ev16 = edge_val.astype(np.float16)
    for c in range(NC):
        m = core_of_edge == c
        e_ids = order[m]
        d = dest[m]
        idx_arr = np.zeros(pad_len, np.int16)
        dloc_arr = np.full(pad_len, 255.0, np.float16)  # pad: no one-hot hit
        val_arr = np.zeros(pad_len, np.float16)
        idx_arr[d] = bidx[e_ids]
        dloc_arr[d] = dloc[e_ids]
        val_arr[d] = ev16[e_ids]
        # per-bank packed gather indices: [128, 8*NBLK_k] int16
        idxps = []
        blk_idx = idx_arr.reshape(NBLK, P)
        for k in range(NBANK):
            sel = blk_idx[bank_of_blk == k].ravel()
            idxps.append(np.tile(
                np.ascontiguousarray(sel.reshape(-1, 16).T), (8, 1)))
        # dv8: per block [dl x8 | val x8]: [128, NBLK, 2, 8] f16
        dl = np.ascontiguousarray(dloc_arr.reshape(NBLK, P).T)  # [128, NBLK]
        vv = np.ascontiguousarray(val_arr.reshape(NBLK, P).T)
        dv8 = np.empty((P, NBLK, 2, 8), np.float16)
        dv8[:, :, 0, :] = dl[:, :, None]
        dv8[:, :, 1, :] = vv[:, :, None]
        cores.append(dict(idxps=idxps, dv8=dv8.reshape(P, NBLK * 16)))

    return nb_tk, v_tk, bko_tk, NBLK_k, cores


def _build_program(nb_tk, v_tk, bko_tk, NBLK_k):
    """Build the SPMD Bass program for the given block structure."""
    nb_tk = np.asarray(nb_tk)
    NB_t = nb_tk.sum(axis=1)
    NBLK = int(NB_t.sum())
    nt = nb_tk.shape[0]
    out_rows = nt * P

    nc = bacc.Bacc("TRN2", target_bir_lowering=False, debug=False,
                   num_devices=NC, num_swdge_queues=4)
    xb_aps = [nc.dram_tensor(f"xb{k}", [BS, D], F16,
                             kind="ExternalInput").ap() for k in range(NBANK)]
    idxp_aps = [nc.dram_tensor(f"idxp{k}", [P, 8 * int(NBLK_k[k])], I16,
                               kind="ExternalInput").ap()
                for k in range(NBANK)]
    dv8_ap = nc.dram_tensor("dv8", [P, 16 * NBLK], F16,
                            kind="ExternalInput").ap()
    w_ap = nc.dram_tensor("w", [D, D], F16, kind="ExternalInput").ap()
    bias_ap = nc.dram_tensor("bias", [P, D], F16, kind="ExternalInput").ap()
    iota_ap = nc.dram_tensor("iota", [P, P], F16, kind="ExternalInput").ap()
    ident_ap = nc.dram_tensor("ident", [P, P], F16, kind="ExternalInput").ap()
    out_ap = nc.dram_tensor("out", [out_rows, D], F32,
                            kind="ExternalOutput").ap()

    nb_max = int(NB_t.max())
    nbk_max = [int(nb_tk[:, k].max()) for k in range(NBANK)]

    with tile.TileContext(nc) as tc:
        with ExitStack() as ctx:
            const = ctx.enter_context(tc.tile_pool(name="const", bufs=1))
            dvpool = ctx.enter_context(tc.tile_pool(name="dvp", bufs=4))
            xgpools = [ctx.enter_context(
                tc.tile_pool(name=f"xgp{k}", bufs=4)) for k in range(NBANK)]
            swpool = ctx.enter_context(tc.tile_pool(name="swp", bufs=3))
            epool = ctx.enter_context(tc.tile_pool(name="ep", bufs=2))
            zpsum = ctx.enter_context(
                tc.tile_pool(name="zps", bufs=4, space="PSUM"))
            tpsum = ctx.enter_context(
                tc.tile_pool(name="tps", bufs=2, space="PSUM"))
            opsum = ctx.enter_context(
                tc.tile_pool(name="ops", bufs=2, space="PSUM"))

            iota_t = const.tile([P, P], F16, tag="iota")
            nc.sync.dma_start(iota_t[:], iota_ap[:])
            ident_t = const.tile([P, P], F16, tag="ident")
            nc.sync.dma_start(ident_t[:], ident_ap[:])
            w_t = const.tile([P, 2, D], F16, tag="w")
            nc.sync.dma_start(w_t[:], w_ap[:].rearrange("(c k) d -> k c d",
                                                        k=P))
            bias_t = const.tile([P, D], F16, tag="bias")
            nc.sync.dma_start(bias_t[:], bias_ap[:])
            idx_consts = []
            for k in range(NBANK):
                ic = const.tile([P, 8 * int(NBLK_k[k])], I16,
                                tag=f"idxc{k}", name=f"idxc{k}")
                nc.sync.dma_start(ic[:], idxp_aps[k][:])
                idx_consts.append(ic)

            TGL = 4  # tiles per idx/dv load group (prefetch + fewer sync ops)
            grp_nb_max = max(int(NB_t[g:g + TGL].sum())
                             for g in range(0, nt, TGL))
            bo = 0   # global block offset
            gbo = 0  # block offset of current group start
            dv_t = None
            for t in range(nt):
                nb = int(NB_t[t])
                if t % TGL == 0:
                    gnb = int(NB_t[t:t + TGL].sum())
                    gbo = bo
                    dv_t = dvpool.tile([P, 16 * grp_nb_max], F16, tag="dv")
                    nc.sync.dma_start(dv_t[:, :16 * gnb],
                                      dv8_ap[:, 16 * bo:16 * (bo + gnb)])
                lo = bo - gbo   # tile's block offset within the group tiles
                xgs = []
                for k in range(NBANK):
                    nbk = int(nb_tk[t, k])
                    xg = xgpools[k].tile([P, nbk_max[k], D], F16,
                                         tag=f"xg{k}", name=f"xg{k}")
                    xgs.append(xg)
                    n = nbk * P
                    ka = int(bko_tk[t, k])
                    nc.gpsimd.dma_gather(
                        out_ap=xg[:, 0:nbk, :],
                        in_ap=xb_aps[k][:],
                        idxs_ap=idx_consts[k][:, 8 * ka:8 * (ka + nbk)],
                        num_idxs=n,
                        num_idxs_reg=n,
                        elem_size=D,
                        # >64 descriptors (~1008 idxs) break the one-packet
                        # ceiling and wedge the exec unit
                        single_packet=(n <= 992),
                        queue_num=k,
                    )

                # batched one-hot build: S[p, j*128+d] = val_j[p] *
                # (iota[d] == dloc_j[p]); two wide DVE tensor_tensor ops.
                s_wide = swpool.tile([P, nb_max * P], F16, tag="sw")
                dv_g = dv_t[:, 16 * lo:16 * (lo + nb)].rearrange(
                    "p (j c r) -> p j c r", c=2, r=8)
                io_b = (iota_t[:].rearrange("p (q r) -> p q r", r=8)
                        .unsqueeze(1).broadcast_to([P, nb, 16, 8]))
                dl_b = dv_g[:, :, 0:1, :].broadcast_to([P, nb, 16, 8])
                vl_b = dv_g[:, :, 1:2, :].broadcast_to([P, nb, 16, 8])
                sw4 = s_wide[:, :nb * P].rearrange(
                    "p (j q r) -> p j q r", q=16, r=8)
                nc.vector.tensor_tensor(out=sw4, in0=io_b, in1=dl_b,
                                        op=mybir.AluOpType.is_equal)
                nc.vector.tensor_tensor(out=sw4, in0=sw4, in1=vl_b,
                                        op=mybir.AluOpType.mult)

                z_ps = zpsum.tile([P, D], F32, tag="zps")
                j = 0
                for k in range(NBANK):
                    nbk = int(nb_tk[t, k])
                    for jl in range(nbk):
                        nc.tensor.matmul(out=z_ps[:],
                                         lhsT=s_wide[:, j * P:(j + 1) * P],
                                         rhs=xgs[k][:, jl, :],
                                         start=(j == 0), stop=(j == nb - 1))
                        j += 1

                z_sb = epool.tile([P, D], F16, tag="zsb")
                nc.scalar.copy(z_sb[:], z_ps[:])
                o_ps = opsum.tile([P, D], F32, tag="ops")
                # seed o_ps with the bias via ident.T @ bias_tile
                nc.tensor.matmul(out=o_ps[:], lhsT=ident_t[:], rhs=bias_t[:],
                                 start=True, stop=False)
                for ch in range(2):
                    zt_ps = tpsum.tile([P, P], F16, tag="ztps")
                    nc.tensor.transpose(zt_ps[:],
                                        z_sb[:, ch * P:(ch + 1) * P],
                                        ident_t[:])
                    zt_sb = epool.tile([P, P], F16, tag="ztsb")
                    nc.scalar.copy(zt_sb[:], zt_ps[:])
                    nc.tensor.matmul(out=o_ps[:], lhsT=zt_sb[:],
                                     rhs=w_t[:, ch, :],
                                     start=False, stop=(ch == 1))
                o_sb = epool.tile([P, D], F32, tag="osb")
                nc.scalar.copy(o_sb[:], o_ps[:])
                nc.sync.dma_start(out_ap[t * P:(t + 1) * P, :], o_sb[:])
                bo += nb
    # The Tile scheduler assigns SWDGE completion sems (DMASW lanes) round-
    # robin in *scheduled* order, and the runtime locks each sem to one
    # SWDGE queue.  Align queue_num with the assigned lane.
    _DMASW0 = 11  # PROC_NAME_TO_IDX["DMASW0"]
    for inst in nc.all_instructions():
        if type(inst).__name__ == "InstDMAGatherAnt":
            inst.queue_num = (inst.bass_scheduled_proc - _DMASW0) % 4
    nc.compile()
    return nc


def kernel(x, edge_row, edge_col, edge_val, weight, b):
    global _last_results
    assert x.shape == (N_NODES, D)

    nb_tk, v_tk, bko_tk, NBLK_k, cores = _build_structure(
        np.asarray(edge_row), np.asarray(edge_col), np.asarray(edge_val))
    nc = _build_program(nb_tk, v_tk, bko_tk, NBLK_k)

    x16 = np.asarray(x, np.float32).astype(np.float16)
    banks = [np.ascontiguousarray(x16[k * BS:(k + 1) * BS])
             for k in range(NBANK)]
    w16 = np.asarray(weight, np.float32).astype(np.float16)
    bias = np.broadcast_to(
        np.asarray(b, np.float32).astype(np.float16)[None, :], (P, D)).copy()
    iota = np.tile(np.arange(P, dtype=np.float16)[None, :], (P, 1))
    ident = np.eye(P, dtype=np.float16)

    in_maps = []
    for c in range(NC):
        m = {f"xb{k}": banks[k] for k in range(NBANK)}
        m.update({f"idxp{k}": cores[c]["idxps"][k] for k in range(NBANK)})
        m.update(dv8=cores[c]["dv8"], w=w16, bias=bias, iota=iota,
                 ident=ident)
        in_maps.append(m)

    trace = bool(os.environ.get("KERNEL_TRACE"))
    res = run_bass_kernel_spmd(nc, in_maps, list(range(NC)), trace=trace)
    _last_results = res

    out = np.concatenate([res.results[c]["out"][:SH] for c in range(NC)],
                         axis=0)
    return out.astype(np.float32)


# revision 10
# speedup vs baseline: 1.4409x; 1.0340x over previous
"""GCNConv kernel for 8 Trainium2 NeuronCores (Bass/Tile).

Computes out = segment_sum(edge_val * (x @ W)[edge_col], edge_row) + b
as out = (A @ x) @ W + b  (associativity), with:
  - nodes (rows of output) sharded across 8 cores (12500 each)
  - edges partitioned by destination row -> per-core, per-128-row-tile
  - per 128-edge block: gather x[col] rows (fp16, 512B) via dma_gather.
    Gathers for the 4 source banks use independent idx tensors and
    output tiles on 4 SWDGE queues so the Tile scheduler cannot
    serialize them.
  - the one-hot scatter matrices S for a whole tile are built with TWO
    batched DVE tensor_tensor ops (is_equal then mult) over a wide
    [128, nb*128] tile.  The comparison operand (dloc) and scale (val)
    are host-replicated x8 so the innermost AP dim stays packed, which
    keeps the DVE in its 2x 16-bit perf mode.
  - z[128 nodes, 256] += S_j.T @ X_j accumulated on the PE in PSUM.
  - epilogue per tile: transpose z, project by W (fp16), bias via a
    PSUM-initializing ident @ bias matmul, store.  All PSUM->SBUF
    copies ride the scalar (ACT) engine; the DVE only does the two
    batched builds per tile.

x is split into 4 banks of 25000 rows because dma_gather indices are
int16.  Padding slots use idx=0 with dloc=255 so their one-hot column
stays zero.
"""
import os
from contextlib import ExitStack

import numpy as np

import concourse.bass as bass
import concourse.tile as tile
from concourse import bacc, mybir
from concourse.bass_utils import run_bass_kernel_spmd

P = 128
D = 256
N_NODES = 100000
N_EDGES = 3200000
NC = 8
SH = N_NODES // NC          # 12500 rows per core
NT = (SH + P - 1) // P      # 98 tiles per core
NBANK = 4
BS = N_NODES // NBANK       # 25000 rows per bank (fits int16 index)

F16 = mybir.dt.float16
F32 = mybir.dt.float32
I16 = mybir.dt.int16

_last_results = None        # BassKernelResults of the most recent run


def _build_structure(edge_row, edge_col, edge_val):
    """Sort/pad edges into per-core 128-edge blocks grouped by
    (dest tile, source bank).  Block structure (nb_tk, v_tk) is shared
    across cores (padded to the max) so one SPMD program fits all cores.

    Returns (nb_tk [NT,NBANK], v_tk [NT,NBANK], per-core dict arrays).
    """
    E = edge_row.shape[0]
    core = edge_row // SH
    r_loc = edge_row - core * SH
    t = r_loc // P
    dloc = (r_loc % P).astype(np.float16)
    bank = edge_col // BS
    bidx = (edge_col % BS).astype(np.int16)

    gid = (core.astype(np.int64) * NT + t) * NBANK + bank
    # secondary sort key: source index, for HBM locality within a group
    order = np.lexsort((bidx, gid))
    gid_s = gid[order]

    cnt = np.bincount(gid, minlength=NC * NT * NBANK).reshape(NC, NT, NBANK)
    v_tk = cnt.max(axis=0)                           # [NT, NBANK] valid idxs
    nb_tk = (v_tk + P - 1) // P                      # [NT, NBANK] blocks
    nb_tk = np.maximum(nb_tk, 1)                     # keep structure non-empty
    NB_t = nb_tk.sum(axis=1)                         # [NT]
    NBLK = int(NB_t.sum())
    pad_len = NBLK * P

    # slot offset of group (t,k) within a core's padded edge list
    off_tk = np.zeros((NT, NBANK), np.int64)
    flat_off = np.cumsum(nb_tk.ravel() * P)
    off_tk.ravel()[1:] = flat_off[:-1]

    # position of each edge within its (c,t,k) group
    grp_start = np.zeros(E, np.int64)
    newgrp = np.ones(E, bool)
    newgrp[1:] = gid_s[1:] != gid_s[:-1]
    starts = np.where(newgrp)[0]
    grp_start[starts] = starts
    grp_start = np.maximum.accumulate(grp_start)
    pos_in_grp = np.arange(E) - grp_start

    tk_of_edge = gid_s % (NT * NBANK)
    core_of_edge = gid_s // (NT * NBANK)
    dest = off_tk.ravel()[tk_of_edge] + pos_in_grp

    # per-bank block bookkeeping: for tile t, bank k, blocks occupy
    # [bko_tk[t,k], bko_tk[t,k]+nb_tk[t,k]) within that bank's block list
    bko_tk = np.zeros((NT, NBANK), np.int64)
    for k in range(NBANK):
        bko_tk[1:, k] = np.cumsum(nb_tk[:-1, k])
    NBLK_k = nb_tk.sum(axis=0)                       # [NBANK]

    # map global block -> (bank, per-bank block id)
    bank_of_blk = np.concatenate(
        [np.repeat(np.arange(NBANK), nb_tk[t]) for t in range(NT)])

    cores = []
    ev16 = edge_val.astype(np.float16)
    for c in range(NC):
        m = core_of_edge == c
        e_ids = order[m]
        d = dest[m]
        idx_arr = np.zeros(pad_len, np.int16)
        dloc_arr = np.full(pad_len, 255.0, np.float16)  # pad: no one-hot hit
        val_arr = np.zeros(pad_len, np.float16)
        idx_arr[d] = bidx[e_ids]
        dloc_arr[d] = dloc[e_ids]
        val_arr[d] = ev16[e_ids]
        # per-bank packed gather indices: [128, 8*NBLK_k] int16
        idxps = []
        blk_idx = idx_arr.reshape(NBLK, P)
        for k in range(NBANK):
            sel = blk_idx[bank_of_blk == k].ravel()
            idxps.append(np.tile(
                np.ascontiguousarray(sel.reshape(-1, 16).T), (8, 1)))
        # dv8: per block [dl x8 | val x8]: [128, NBLK, 2, 8] f16
        dl = np.ascontiguousarray(dloc_arr.reshape(NBLK, P).T)  # [128, NBLK]
        vv = np.ascontiguousarray(val_arr.reshape(NBLK, P).T)
        dv8 = np.empty((P, NBLK, 2, 8), np.float16)
        dv8[:, :, 0, :] = dl[:, :, None]
        dv8[:, :, 1, :] = vv[:, :, None]
        cores.append(dict(idxps=idxps, dv8=dv8.reshape(P, NBLK * 16)))

    return nb_tk, v_tk, bko_tk, NBLK_k, cores


def _build_program(nb_tk, v_tk, bko_tk, NBLK_k):
    """Build the SPMD Bass program for the given block structure."""
    nb_tk = np.asarray(nb_tk)
    NB_t = nb_tk.sum(axis=1)
    NBLK = int(NB_t.sum())
    nt = nb_tk.shape[0]
    out_rows = nt * P

    nc = bacc.Bacc("TRN2", target_bir_lowering=False, debug=False,
                   num_devices=NC, num_swdge_queues=4)
    xb_aps = [nc.dram_tensor(f"xb{k}", [BS, D], F16,
                             kind="ExternalInput").ap() for k in range(NBANK)]
    idxp_aps = [nc.dram_tensor(f"idxp{k}", [P, 8 * int(NBLK_k[k])], I16,
                               kind="ExternalInput").ap()
                for k in range(NBANK)]
    dv8_ap = nc.dram_tensor("dv8", [P, 16 * NBLK], F16,
                            kind="ExternalInput").ap()
    w_ap = nc.dram_tensor("w", [D, D], F16, kind="ExternalInput").ap()
    bias_ap = nc.dram_tensor("bias", [P, D], F16, kind="ExternalInput").ap()
    iota_ap = nc.dram_tensor("iota", [P, P], F16, kind="ExternalInput").ap()
    ident_ap = nc.dram_tensor("ident", [P, P], F16, kind="ExternalInput").ap()
    out_ap = nc.dram_tensor("out", [out_rows, D], F32,
                            kind="ExternalOutput").ap()

    nb_max = int(NB_t.max())
    nbk_max = [int(nb_tk[:, k].max()) for k in range(NBANK)]

    with tile.TileContext(nc) as tc:
        with ExitStack() as ctx:
            const = ctx.enter_context(tc.tile_pool(name="const", bufs=1))
            dvpool = ctx.enter_context(tc.tile_pool(name="dvp", bufs=4))
            xgpools = [ctx.enter_context(
                tc.tile_pool(name=f"xgp{k}", bufs=4)) for k in range(NBANK)]
            swpool = ctx.enter_context(tc.tile_pool(name="swp", bufs=3))
            epool = ctx.enter_context(tc.tile_pool(name="ep", bufs=2))
            zpsum = ctx.enter_context(
                tc.tile_pool(name="zps", bufs=4, space="PSUM"))
            tpsum = ctx.enter_context(
                tc.tile_pool(name="tps", bufs=2, space="PSUM"))
            opsum = ctx.enter_context(
                tc.tile_pool(name="ops", bufs=2, space="PSUM"))

            iota_t = const.tile([P, P], F16, tag="iota")
            nc.sync.dma_start(iota_t[:], iota_ap[:])
            ident_t = const.tile([P, P], F16, tag="ident")
            nc.sync.dma_start(ident_t[:], ident_ap[:])
            w_t = const.tile([P, 2, D], F16, tag="w")
            nc.sync.dma_start(w_t[:], w_ap[:].rearrange("(c k) d -> k c d",
                                                        k=P))
            bias_t = const.tile([P, D], F16, tag="bias")
            nc.sync.dma_start(bias_t[:], bias_ap[:])
            idx_consts = []
            for k in range(NBANK):
                ic = const.tile([P, 8 * int(NBLK_k[k])], I16,
                                tag=f"idxc{k}", name=f"idxc{k}")
                nc.sync.dma_start(ic[:], idxp_aps[k][:])
                idx_consts.append(ic)

            # zero each physical xg buffer once: gather slots beyond the
            # shared valid count v are never written (masked out of the
            # gather), and 0 * NaN would poison the matmul
            for k in range(NBANK):
                for _r in range(4):
                    xgi = xgpools[k].tile([P, nbk_max[k], D], F16,
                                          tag=f"xg{k}", name=f"xg{k}i")
                    nc.vector.memset(xgi[:], 0.0)

            TGL = 4  # tiles per idx/dv load group (prefetch + fewer sync ops)
            grp_nb_max = max(int(NB_t[g:g + TGL].sum())
                             for g in range(0, nt, TGL))
            bo = 0   # global block offset
            gbo = 0  # block offset of current group start
            dv_t = None
            for t in range(nt):
                nb = int(NB_t[t])
                if t % TGL == 0:
                    gnb = int(NB_t[t:t + TGL].sum())
                    gbo = bo
                    dv_t = dvpool.tile([P, 16 * grp_nb_max], F16, tag="dv")
                    nc.sync.dma_start(dv_t[:, :16 * gnb],
                                      dv8_ap[:, 16 * bo:16 * (bo + gnb)])
                lo = bo - gbo   # tile's block offset within the group tiles
                xgs = []
                for k in range(NBANK):
                    nbk = int(nb_tk[t, k])
                    xg = xgpools[k].tile([P, nbk_max[k], D], F16,
                                         tag=f"xg{k}", name=f"xg{k}")
                    xgs.append(xg)
                    n = nbk * P
                    v = int(v_tk[t, k])
                    ka = int(bko_tk[t, k])
                    nc.gpsimd.dma_gather(
                        out_ap=xg[:, 0:nbk, :],
                        in_ap=xb_aps[k][:],
                        idxs_ap=idx_consts[k][:, 8 * ka:8 * (ka + nbk)],
                        num_idxs=v,
                        num_idxs_reg=v,
                        elem_size=D,
                        # >64 descriptors (~1008 idxs) break the one-packet
                        # ceiling and wedge the exec unit
                        single_packet=(n <= 992),
                        queue_num=k,
                    )

                # batched one-hot build: S[p, j*128+d] = val_j[p] *
                # (iota[d] == dloc_j[p]); two wide DVE tensor_tensor ops.
                s_wide = swpool.tile([P, nb_max * P], F16, tag="sw")
                dv_g = dv_t[:, 16 * lo:16 * (lo + nb)].rearrange(
                    "p (j c r) -> p j c r", c=2, r=8)
                io_b = (iota_t[:].rearrange("p (q r) -> p q r", r=8)
                        .unsqueeze(1).broadcast_to([P, nb, 16, 8]))
                dl_b = dv_g[:, :, 0:1, :].broadcast_to([P, nb, 16, 8])
                vl_b = dv_g[:, :, 1:2, :].broadcast_to([P, nb, 16, 8])
                sw4 = s_wide[:, :nb * P].rearrange(
                    "p (j q r) -> p j q r", q=16, r=8)
                nc.vector.tensor_tensor(out=sw4, in0=io_b, in1=dl_b,
                                        op=mybir.AluOpType.is_equal)
                nc.vector.tensor_tensor(out=sw4, in0=sw4, in1=vl_b,
                                        op=mybir.AluOpType.mult)

                z_ps = zpsum.tile([P, D], F32, tag="zps")
                j = 0
                for k in range(NBANK):
                    nbk = int(nb_tk[t, k])
                    for jl in range(nbk):
                        nc.tensor.matmul(out=z_ps[:],
                                         lhsT=s_wide[:, j * P:(j + 1) * P],
                                         rhs=xgs[k][:, jl, :],
                                         start=(j == 0), stop=(j == nb - 1))
                        j += 1

                z_sb = epool.tile([P, D], F16, tag="zsb")
                nc.scalar.copy(z_sb[:], z_ps[:])
                o_ps = opsum.tile([P, D], F32, tag="ops")
                # seed o_ps with the bias via ident.T @ bias_tile
                nc.tensor.matmul(out=o_ps[:], lhsT=ident_t[:], rhs=bias_t[:],
                                 start=True, stop=False)
                for ch in range(2):
                    zt_ps = tpsum.tile([P, P], F16, tag="ztps")
                    nc.tensor.transpose(zt_ps[:],
                                        z_sb[:, ch * P:(ch + 1) * P],
                                        ident_t[:])
                    zt_sb = epool.tile([P, P], F16, tag="ztsb")
                    nc.scalar.copy(zt_sb[:], zt_ps[:])
                    nc.tensor.matmul(out=o_ps[:], lhsT=zt_sb[:],
                                     rhs=w_t[:, ch, :],
                                     start=False, stop=(ch == 1))
                o_sb = epool.tile([P, D], F32, tag="osb")
                nc.scalar.copy(o_sb[:], o_ps[:])
                nc.sync.dma_start(out_ap[t * P:(t + 1) * P, :], o_sb[:])
                bo += nb
    # The Tile scheduler assigns SWDGE completion sems (DMASW lanes) round-
    # robin in *scheduled* order, and the runtime locks each sem to one
    # SWDGE queue.  Align queue_num with the assigned lane.
    _DMASW0 = 11  # PROC_NAME_TO_IDX["DMASW0"]
    for inst in nc.all_instructions():
        if type(inst).__name__ == "InstDMAGatherAnt":
            inst.queue_num = (inst.bass_scheduled_proc - _DMASW0) % 4
    nc.compile()
    return nc


def kernel(x, edge_row, edge_col, edge_val, weight, b):
    global _last_results
    assert x.shape == (N_NODES, D)

    nb_tk, v_tk, bko_tk, NBLK_k, cores = _build_structure(
        np.asarray(edge_row), np.asarray(edge_col), np.asarray(edge_val))
    nc = _build_program(nb_tk, v_tk, bko_tk, NBLK_k)

    x16 = np.asarray(x, np.float32).astype(np.float16)
    banks = [np.ascontiguousarray(x16[k * BS:(k + 1) * BS])
             for k in range(NBANK)]
    w16 = np.asarray(weight, np.float32).astype(np.float16)
    bias = np.broadcast_to(
        np.asarray(b, np.float32).astype(np.float16)[None, :], (P, D)).copy()
    iota = np.tile(np.arange(P, dtype=np.float16)[None, :], (P, 1))
    ident = np.eye(P, dtype=np.float16)

    in_maps = []
    for c in range(NC):
        m = {f"xb{k}": banks[k] for k in range(NBANK)}
        m.update({f"idxp{k}": cores[c]["idxps"][k] for k in range(NBANK)})
        m.update(dv8=cores[c]["dv8"], w=w16, bias=bias, iota=iota,
                 ident=ident)
        in_maps.append(m)

    trace = bool(os.environ.get("KERNEL_TRACE"))
    res = run_bass_kernel_spmd(nc, in_maps, list(range(NC)), trace=trace)
    _last_results = res

    out = np.concatenate([res.results[c]["out"][:SH] for c in range(NC)],
                         axis=0)
    return out.astype(np.float32)
